# revision 7
# baseline (speedup 1.0000x reference)
"""Paged-KV GQA decode attention on 8 TRN2 NeuronCores.

Strategy (data-parallel over flattened token tiles):
  * Host: resolve the paged cache (block_tables is a disjoint contiguous
    arange layout -> zero-copy reshape; general gather fallback otherwise),
    apply the store_kvcache update, slice each sequence's valid prefix
    [0, ctx_len), pad to 128-token tiles, and pack the global tile list.
  * The global tile stream is split contiguously across the 8 cores
    (perfect +-1 tile balance). Per tile the device computes, for each of
    the 8 KV heads, scoresT = K_tile^T @ qT (PE, stationary = K^T so scores
    land transposed [s, q]), w = exp(scoresT) (ACT, no max subtraction
    needed: |scores| <= ~6), o_tile = V_tile^T @ w (PE), and
    l_tile = ones^T @ w (PE). Per-tile unnormalized (o, l) go back to HBM.
  * Host: sum (o, l) over each sequence's tiles, subtract the exp(0)=1
    contribution of the zero-padded slots from l, divide, transpose.

Layouts are pre-transposed on the host so every device DMA is one fully
contiguous block per tile and the PE never needs an on-chip transpose.
Per-tile input row layout (128 partitions x 2080 bf16):
  cols [0,1024):     K^T   (partition=d, col=kh*128+s)
  cols [1024,2048):  V     (partition=s, col=kh*128+d)
  cols [2048,2080):  q^T   (partition=d, col=kh*4+j), pre-scaled by 1/sqrt(D)
Per-tile output row layout (128 partitions x 64 f32):
  cols [0,32):  o_tile (partition=d, col=kh*4+j), unnormalized
  row 0, cols [32,64):  l_tile (sum of exp weights per (kh,j))
"""

import math
import os

import numpy as np

B, H, KVH, D = 32, 32, 8, 128
G = H // KVH
BLOCK_SIZE = 16
MAX_BLOCKS = 256
NUM_BLOCKS = B * MAX_BLOCKS
MAX_KV = MAX_BLOCKS * BLOCK_SIZE
SCALE = 0.08838834764831845
NCORES = 8
TILE = 128

KV_DTYPE = os.environ.get("BASS_KV_DTYPE", "bfloat16")

X_COLS = KVH * TILE + KVH * D + H  # 2080
O_OFF = 0
L_OFF = H  # in the [128, 64] output tile, l lives at row 0, cols [32,64)

LAST_RESULT = None  # BassKernelResults of the most recent run (for test.py)

_NC_CACHE = {}


def _install_trace_shim():
    """Register the axon NTFF profile hook (missing from the stub antenv) and
    stub the S3 artifact upload, so trace=True yields exec_time_ns."""
    import sys
    import types

    if "antenv.axon_hooks" not in sys.modules:
        mod = types.ModuleType("antenv.axon_hooks")
        _hook = [None]
        mod.set_axon_ntff_profile_hook = lambda h: _hook.__setitem__(0, h)
        mod.get_axon_ntff_profile_hook = lambda: _hook[0]
        sys.modules["antenv.axon_hooks"] = mod
        import antenv

        antenv.axon_hooks = mod
    from antenv.axon_hooks import (
        get_axon_ntff_profile_hook,
        set_axon_ntff_profile_hook,
    )

    if get_axon_ntff_profile_hook() is None:
        try:
            from trn_agent_boot.trn_boot import _ntff_profile_via_ctypes

            set_axon_ntff_profile_hook(
                _ntff_profile_via_ctypes("/opt/axon/libaxon_pjrt.so")
            )
        except Exception:
            pass
    import concourse.bass_utils as bu

    bu.upload_artifacts = lambda tmpdir: f"file://{tmpdir}"


def _install_ldw_opt_patch():
    """Experiment: flip walrus --enable-ldw-opt to true (default pipeline passes
    false). Gated by BASS_LDW_OPT=1."""
    import concourse.bass_utils as bu

    orig = bu.run_command

    def patched(argv, **kwargs):
        argv = [
            a.replace("--enable-ldw-opt=false", "--enable-ldw-opt=true")
            if isinstance(a, str)
            else a
            for a in argv
        ]
        return orig(argv, **kwargs)

    if getattr(bu.run_command, "__name__", "") != "patched":
        bu.run_command = patched


def _build_nc(n_t: int, dt_name: str):
    import concourse.mybir as mybir
    import concourse.tile as tile
    from concourse import bacc

    key = (n_t, dt_name)
    if key in _NC_CACHE:
        return _NC_CACHE[key]

    DT = getattr(mybir.dt, dt_name)
    F32 = mybir.dt.float32
    KOFF, VOFF, QOFF = 0, KVH * TILE, 2 * KVH * TILE

    nc = bacc.Bacc("TRN2", target_bir_lowering=False, num_devices=NCORES)
    x = nc.dram_tensor("x", [n_t, TILE, X_COLS], DT, kind="ExternalInput")
    y = nc.dram_tensor("y", [n_t, TILE, 2 * H], F32, kind="ExternalOutput")

    with tile.TileContext(nc) as tc:
        with (
            tc.tile_pool(name="consts", bufs=1) as consts,
            tc.tile_pool(name="xp", bufs=6) as x_pool,
            tc.tile_pool(name="wt", bufs=4) as wt_pool,
            tc.tile_pool(name="outs", bufs=4) as out_pool,
            tc.tile_pool(name="ps_sc", bufs=3, space="PSUM") as ps_sc,
            tc.tile_pool(name="ps_o", bufs=3, space="PSUM") as ps_o,
            tc.tile_pool(name="ps_l", bufs=2, space="PSUM") as ps_l,
        ):
            ones = consts.tile([TILE, 1], DT)
            nc.vector.memset(ones, 1.0)

            def emit_pv(x_t, w_t, t):
                o_ps = ps_o.tile([D, H], F32)
                for kh in range(KVH):
                    nc.tensor.matmul(
                        o_ps[:, kh * G:(kh + 1) * G],
                        lhsT=x_t[:, VOFF + kh * D:VOFF + (kh + 1) * D],
                        rhs=w_t[:, kh * G:(kh + 1) * G],
                        start=(kh == 0),
                        stop=(kh == KVH - 1),
                    )
                l_ps = ps_l.tile([1, H], F32)
                nc.tensor.matmul(l_ps, lhsT=ones, rhs=w_t, start=True, stop=True)

                y_sb = out_pool.tile([TILE, 2 * H], F32)
                nc.vector.tensor_copy(y_sb[:, :H], o_ps)
                nc.vector.tensor_copy(y_sb[0:1, H:], l_ps)
                nc.sync.dma_start(out=y[t], in_=y_sb)

            prev = None
            for t in range(n_t):
                x_t = x_pool.tile([TILE, X_COLS], DT)
                nc.sync.dma_start(out=x_t, in_=x[t])

                sc = ps_sc.tile([TILE, H], F32)
                for kh in range(KVH):
                    nc.tensor.matmul(
                        sc[:, kh * G:(kh + 1) * G],
                        lhsT=x_t[:, KOFF + kh * TILE:KOFF + (kh + 1) * TILE],
                        rhs=x_t[:, QOFF + kh * G:QOFF + (kh + 1) * G],
                        start=(kh == 0),
                        stop=(kh == KVH - 1),
                    )
                w_t = wt_pool.tile([TILE, H], DT)
                nc.scalar.activation(w_t, sc, mybir.ActivationFunctionType.Exp)

                # software pipeline: PV for tile t-1 runs while exp(t) is on ACT,
                # so the PE never stalls on the QK->exp->PV chain of one tile.
                if prev is not None:
                    emit_pv(*prev)
                prev = (x_t, w_t, t)
            emit_pv(*prev)
    nc.finalize()
    _NC_CACHE[key] = nc
    return nc


def kernel(q, k, v, k_cache, v_cache, block_tables, context_lens, slot_mapping):
    global LAST_RESULT
    from concourse.bass_utils import run_bass_kernel_spmd

    trace = bool(os.environ.get("BASS_TRACE"))
    if trace:
        _install_trace_shim()
    if os.environ.get("BASS_LDW_OPT"):
        _install_ldw_opt_patch()

    q = np.asarray(q, dtype=np.float32)
    k = np.asarray(k, dtype=np.float32)
    v = np.asarray(v, dtype=np.float32)
    k_cache = np.asarray(k_cache)
    v_cache = np.asarray(v_cache)
    block_tables = np.asarray(block_tables)
    context_lens = np.asarray(context_lens).astype(np.int64)
    slot_mapping = np.asarray(slot_mapping).astype(np.int64)

    # --- resolve paged layout -------------------------------------------------
    if np.array_equal(block_tables.ravel(), np.arange(NUM_BLOCKS, dtype=np.int64)):
        k_seq = k_cache.reshape(B, MAX_KV, KVH, D)  # zero-copy view
        v_seq = v_cache.reshape(B, MAX_KV, KVH, D)
        flat_pos = slot_mapping  # slot index == b*MAX_KV + pos under arange tables
    else:  # general fallback: true gather (slow, but correct for any table)
        k_seq = k_cache[block_tables].reshape(B, MAX_KV, KVH, D)
        v_seq = v_cache[block_tables].reshape(B, MAX_KV, KVH, D)
        blk = slot_mapping // BLOCK_SIZE
        off = slot_mapping % BLOCK_SIZE
        flat_pos = np.empty(B, np.int64)
        for b in range(B):
            tb = np.where(block_tables[b] == blk[b])[0][0]
            flat_pos[b] = b * MAX_KV + tb * BLOCK_SIZE + off[b]

    # --- tile map -------------------------------------------------------------
    ctx = context_lens.astype(np.int64)
    n_t_seq = [int(math.ceil(int(c) / TILE)) for c in ctx]
    seq_tile_start = np.concatenate([[0], np.cumsum(n_t_seq)]).astype(np.int64)
    g_tiles = int(seq_tile_start[-1])
    n_t = (g_tiles + NCORES - 1) // NCORES
    g_pad = n_t * NCORES

    if KV_DTYPE == "bfloat16":
        import ml_dtypes

        dt_np = ml_dtypes.bfloat16
    else:
        dt_np = np.float32

    x_g = np.zeros((g_pad, TILE, X_COLS), dt_np)
    KOFF, VOFF, QOFF = 0, KVH * TILE, 2 * KVH * TILE

    for b in range(B):
        c = int(ctx[b])
        t0 = int(seq_tile_start[b])
        nt = n_t_seq[b]
        kb = np.zeros((nt * TILE, KVH, D), np.float32)
        vb = np.zeros((nt * TILE, KVH, D), np.float32)
        kb[:c] = k_seq[b, :c]
        vb[:c] = v_seq[b, :c]
        # store_kvcache: new token for seq b lands at flat_pos[b] % MAX_KV
        p = int(flat_pos[b] - b * MAX_KV)
        if 0 <= p < c:
            kb[p] = k[b]
            vb[p] = v[b]
        # K^T tiles: [s, kh, d] -> [t, d, kh, s]
        kt = kb.reshape(nt, TILE, KVH, D).transpose(0, 3, 2, 1)
        x_g[t0:t0 + nt, :, KOFF:VOFF] = kt.reshape(nt, D, KVH * TILE).astype(dt_np)
        # V tiles: [t, s, kh*d]
        x_g[t0:t0 + nt, :, VOFF:QOFF] = vb.reshape(nt, TILE, KVH * D).astype(dt_np)
        x_g[t0:t0 + nt, :, QOFF:] = (q[b].T * SCALE).astype(dt_np)[None]

    in_maps = [{"x": x_g[c0 * n_t:(c0 + 1) * n_t]} for c0 in range(NCORES)]

    nc = _build_nc(n_t, KV_DTYPE)
    res = run_bass_kernel_spmd(
        nc, in_maps, core_ids=list(range(NCORES)), trace=trace
    )
    LAST_RESULT = res

    y_all = np.concatenate([res.results[c]["y"] for c in range(NCORES)], axis=0)

    out = np.empty((B, H, D), np.float32)
    for b in range(B):
        t0 = int(seq_tile_start[b])
        nt = n_t_seq[b]
        o_b = y_all[t0:t0 + nt, :, :H].sum(axis=0)       # [D, H]
        l_b = y_all[t0:t0 + nt, 0, H:].sum(axis=0)       # [H]
        l_b = l_b - (nt * TILE - int(ctx[b]))            # remove exp(0) pad terms
        out[b] = (o_b / l_b).T
    return out


# revision 11
# speedup vs baseline: 1.3165x; 1.3165x over previous
"""Paged-KV GQA decode attention on 8 TRN2 NeuronCores.

Strategy (data-parallel over flattened token tiles):
  * Host: resolve the paged cache (block_tables is a disjoint contiguous
    arange layout -> zero-copy reshape; general gather fallback otherwise),
    apply the store_kvcache update, slice each sequence's valid prefix
    [0, ctx_len), pad to 128-token tiles, and pack the global tile list.
  * The global tile stream is split contiguously across the 8 cores
    (perfect +-1 tile balance). Per tile the device computes, for each of
    the 8 KV heads, scoresT = K_tile^T @ qT (PE, stationary = K^T so scores
    land transposed [s, q]), w = exp(scoresT) (ACT, no max subtraction
    needed: |scores| <= ~6), o_tile = V_tile^T @ w (PE), and
    l_tile = ones^T @ w (PE). Per-tile unnormalized (o, l) go back to HBM.
  * Host: sum (o, l) over each sequence's tiles, subtract the exp(0)=1
    contribution of the zero-padded slots from l, divide, transpose.

Layouts are pre-transposed on the host so every device DMA is one fully
contiguous block per tile and the PE never needs an on-chip transpose.
Per-tile input row layout (128 partitions x 2080 bf16):
  cols [0,1024):     K^T   (partition=d, col=kh*128+s)
  cols [1024,2048):  V     (partition=s, col=kh*128+d)
  cols [2048,2080):  q^T   (partition=d, col=kh*4+j), pre-scaled by 1/sqrt(D)
Per-tile output row layout (128 partitions x 64 f32):
  cols [0,32):  o_tile (partition=d, col=kh*4+j), unnormalized
  row 0, cols [32,64):  l_tile (sum of exp weights per (kh,j))
"""

import math
import os

import numpy as np

B, H, KVH, D = 32, 32, 8, 128
G = H // KVH
BLOCK_SIZE = 16
MAX_BLOCKS = 256
NUM_BLOCKS = B * MAX_BLOCKS
MAX_KV = MAX_BLOCKS * BLOCK_SIZE
SCALE = 0.08838834764831845
NCORES = 8
TILE = 128

KV_DTYPE = os.environ.get("BASS_KV_DTYPE", "bfloat16")

X_COLS = KVH * TILE + KVH * D + H  # 2080
O_OFF = 0
L_OFF = H  # in the [128, 64] output tile, l lives at row 0, cols [32,64)

LAST_RESULT = None  # BassKernelResults of the most recent run (for test.py)

_NC_CACHE = {}


def _install_trace_shim():
    """Register the axon NTFF profile hook (missing from the stub antenv) and
    stub the S3 artifact upload, so trace=True yields exec_time_ns."""
    import sys
    import types

    if "antenv.axon_hooks" not in sys.modules:
        mod = types.ModuleType("antenv.axon_hooks")
        _hook = [None]
        mod.set_axon_ntff_profile_hook = lambda h: _hook.__setitem__(0, h)
        mod.get_axon_ntff_profile_hook = lambda: _hook[0]
        sys.modules["antenv.axon_hooks"] = mod
        import antenv

        antenv.axon_hooks = mod
    from antenv.axon_hooks import (
        get_axon_ntff_profile_hook,
        set_axon_ntff_profile_hook,
    )

    if get_axon_ntff_profile_hook() is None:
        try:
            from trn_agent_boot.trn_boot import _ntff_profile_via_ctypes

            set_axon_ntff_profile_hook(
                _ntff_profile_via_ctypes("/opt/axon/libaxon_pjrt.so")
            )
        except Exception:
            pass
    import concourse.bass_utils as bu

    bu.upload_artifacts = lambda tmpdir: f"file://{tmpdir}"


def _install_ldw_opt_patch():
    """Experiment: flip walrus --enable-ldw-opt to true (default pipeline passes
    false). Gated by BASS_LDW_OPT=1."""
    import concourse.bass_utils as bu

    orig = bu.run_command

    def patched(argv, **kwargs):
        argv = [
            a.replace("--enable-ldw-opt=false", "--enable-ldw-opt=true")
            if isinstance(a, str)
            else a
            for a in argv
        ]
        return orig(argv, **kwargs)

    if getattr(bu.run_command, "__name__", "") != "patched":
        bu.run_command = patched


def _build_nc(n_t: int, dt_name: str):
    import concourse.mybir as mybir
    import concourse.tile as tile
    from concourse import bacc

    key = (n_t, dt_name)
    if key in _NC_CACHE:
        return _NC_CACHE[key]

    DT = getattr(mybir.dt, dt_name)
    F32 = mybir.dt.float32
    KOFF, VOFF, QOFF = 0, KVH * TILE, 2 * KVH * TILE

    assert n_t % 4 == 0
    nc = bacc.Bacc("TRN2", target_bir_lowering=False, num_devices=NCORES)
    x = nc.dram_tensor("x", [n_t, TILE, X_COLS], DT, kind="ExternalInput")
    # y batches 4 tiles per row-block so output DMA rows are 1 KiB, not 256 B
    y = nc.dram_tensor("y", [n_t // 4, TILE, 8 * H], F32, kind="ExternalOutput")

    with tile.TileContext(nc) as tc:
        with (
            tc.tile_pool(name="consts", bufs=1) as consts,
            tc.tile_pool(name="xp", bufs=6) as x_pool,
            tc.tile_pool(name="wt", bufs=4) as wt_pool,
            tc.tile_pool(name="outs", bufs=4) as out_pool,
            tc.tile_pool(name="ps_sc", bufs=3, space="PSUM") as ps_sc,
            tc.tile_pool(name="ps_o", bufs=3, space="PSUM") as ps_o,
            tc.tile_pool(name="ps_l", bufs=2, space="PSUM") as ps_l,
        ):
            ones = consts.tile([TILE, 1], DT)
            nc.vector.memset(ones, 1.0)

            y_sb = None
            for t in range(n_t):
                x_t = x_pool.tile([TILE, X_COLS], DT)
                # alternate the two HWDGE rings (SP / ACT) to keep the 16 SDMA
                # engines continuously fed
                dma_eng = nc.sync if t % 2 == 0 else nc.scalar
                dma_eng.dma_start(out=x_t, in_=x[t])

                sc = ps_sc.tile([TILE, H], F32)
                for kh in range(KVH):
                    nc.tensor.matmul(
                        sc[:, kh * G:(kh + 1) * G],
                        lhsT=x_t[:, KOFF + kh * TILE:KOFF + (kh + 1) * TILE],
                        rhs=x_t[:, QOFF + kh * G:QOFF + (kh + 1) * G],
                        start=(kh == 0),
                        stop=(kh == KVH - 1),
                    )
                w_t = wt_pool.tile([TILE, H], DT)
                nc.scalar.activation(w_t, sc, mybir.ActivationFunctionType.Exp)

                o_ps = ps_o.tile([D, H], F32)
                for kh in range(KVH):
                    nc.tensor.matmul(
                        o_ps[:, kh * G:(kh + 1) * G],
                        lhsT=x_t[:, VOFF + kh * D:VOFF + (kh + 1) * D],
                        rhs=w_t[:, kh * G:(kh + 1) * G],
                        start=(kh == 0),
                        stop=(kh == KVH - 1),
                    )
                l_ps = ps_l.tile([1, H], F32)
                nc.tensor.matmul(l_ps, lhsT=ones, rhs=w_t, start=True, stop=True)

                if t % 4 == 0:
                    y_sb = out_pool.tile([TILE, 8 * H], F32)
                off = (t % 4) * 2 * H
                nc.vector.tensor_copy(y_sb[:, off:off + H], o_ps)
                nc.vector.tensor_copy(y_sb[0:1, off + H:off + 2 * H], l_ps)
                if t % 4 == 3:
                    nc.sync.dma_start(out=y[t // 4], in_=y_sb)
    nc.finalize()
    _NC_CACHE[key] = nc
    return nc


def kernel(q, k, v, k_cache, v_cache, block_tables, context_lens, slot_mapping):
    global LAST_RESULT
    from concourse.bass_utils import run_bass_kernel_spmd

    trace = bool(os.environ.get("BASS_TRACE"))
    if trace:
        _install_trace_shim()
    if os.environ.get("BASS_LDW_OPT"):
        _install_ldw_opt_patch()

    q = np.asarray(q, dtype=np.float32)
    k = np.asarray(k, dtype=np.float32)
    v = np.asarray(v, dtype=np.float32)
    k_cache = np.asarray(k_cache)
    v_cache = np.asarray(v_cache)
    block_tables = np.asarray(block_tables)
    context_lens = np.asarray(context_lens).astype(np.int64)
    slot_mapping = np.asarray(slot_mapping).astype(np.int64)

    # --- resolve paged layout -------------------------------------------------
    if np.array_equal(block_tables.ravel(), np.arange(NUM_BLOCKS, dtype=np.int64)):
        k_seq = k_cache.reshape(B, MAX_KV, KVH, D)  # zero-copy view
        v_seq = v_cache.reshape(B, MAX_KV, KVH, D)
        flat_pos = slot_mapping  # slot index == b*MAX_KV + pos under arange tables
    else:  # general fallback: true gather (slow, but correct for any table)
        k_seq = k_cache[block_tables].reshape(B, MAX_KV, KVH, D)
        v_seq = v_cache[block_tables].reshape(B, MAX_KV, KVH, D)
        blk = slot_mapping // BLOCK_SIZE
        off = slot_mapping % BLOCK_SIZE
        flat_pos = np.empty(B, np.int64)
        for b in range(B):
            tb = np.where(block_tables[b] == blk[b])[0][0]
            flat_pos[b] = b * MAX_KV + tb * BLOCK_SIZE + off[b]

    # --- tile map -------------------------------------------------------------
    ctx = context_lens.astype(np.int64)
    n_t_seq = [int(math.ceil(int(c) / TILE)) for c in ctx]
    seq_tile_start = np.concatenate([[0], np.cumsum(n_t_seq)]).astype(np.int64)
    g_tiles = int(seq_tile_start[-1])
    n_t = (g_tiles + NCORES - 1) // NCORES
    n_t = (n_t + 3) // 4 * 4  # y-batching works in groups of 4 tiles
    g_pad = n_t * NCORES

    if KV_DTYPE == "bfloat16":
        import ml_dtypes

        dt_np = ml_dtypes.bfloat16
    else:
        dt_np = np.float32

    x_g = np.zeros((g_pad, TILE, X_COLS), dt_np)
    KOFF, VOFF, QOFF = 0, KVH * TILE, 2 * KVH * TILE

    for b in range(B):
        c = int(ctx[b])
        t0 = int(seq_tile_start[b])
        nt = n_t_seq[b]
        kb = np.zeros((nt * TILE, KVH, D), np.float32)
        vb = np.zeros((nt * TILE, KVH, D), np.float32)
        kb[:c] = k_seq[b, :c]
        vb[:c] = v_seq[b, :c]
        # store_kvcache: new token for seq b lands at flat_pos[b] % MAX_KV
        p = int(flat_pos[b] - b * MAX_KV)
        if 0 <= p < c:
            kb[p] = k[b]
            vb[p] = v[b]
        # K^T tiles: [s, kh, d] -> [t, d, kh, s]
        kt = kb.reshape(nt, TILE, KVH, D).transpose(0, 3, 2, 1)
        x_g[t0:t0 + nt, :, KOFF:VOFF] = kt.reshape(nt, D, KVH * TILE).astype(dt_np)
        # V tiles: [t, s, kh*d]
        x_g[t0:t0 + nt, :, VOFF:QOFF] = vb.reshape(nt, TILE, KVH * D).astype(dt_np)
        x_g[t0:t0 + nt, :, QOFF:] = (q[b].T * SCALE).astype(dt_np)[None]

    in_maps = [{"x": x_g[c0 * n_t:(c0 + 1) * n_t]} for c0 in range(NCORES)]

    nc = _build_nc(n_t, KV_DTYPE)
    res = run_bass_kernel_spmd(
        nc, in_maps, core_ids=list(range(NCORES)), trace=trace
    )
    LAST_RESULT = res

    y_b = np.concatenate([res.results[c]["y"] for c in range(NCORES)], axis=0)
    # [G4, 128, 4*64] -> per-tile [G, 128, 64]
    y_all = (
        y_b.reshape(-1, TILE, 4, 2 * H).transpose(0, 2, 1, 3).reshape(-1, TILE, 2 * H)
    )

    out = np.empty((B, H, D), np.float32)
    for b in range(B):
        t0 = int(seq_tile_start[b])
        nt = n_t_seq[b]
        o_b = y_all[t0:t0 + nt, :, :H].sum(axis=0)       # [D, H]
        l_b = y_all[t0:t0 + nt, 0, H:].sum(axis=0)       # [H]
        l_b = l_b - (nt * TILE - int(ctx[b]))            # remove exp(0) pad terms
        out[b] = (o_b / l_b).T
    return out


# revision 16
# speedup vs baseline: 1.3398x; 1.0177x over previous
"""Paged-KV GQA decode attention on 8 TRN2 NeuronCores.

Strategy (data-parallel over flattened token tiles):
  * Host: resolve the paged cache (block_tables is a disjoint contiguous
    arange layout -> zero-copy reshape; general gather fallback otherwise),
    apply the store_kvcache update, slice each sequence's valid prefix
    [0, ctx_len), pad to 128-token tiles, and pack the global tile list.
  * The global tile stream is split contiguously across the 8 cores
    (perfect +-1 tile balance). Per tile the device computes, for each of
    the 8 KV heads, scoresT = K_tile^T @ qT (PE, stationary = K^T so scores
    land transposed [s, q]), w = exp(scoresT) (ACT, no max subtraction
    needed: |scores| <= ~6), o_tile = V_tile^T @ w (PE), and
    l_tile = ones^T @ w (PE). Per-tile unnormalized (o, l) go back to HBM.
  * Host: sum (o, l) over each sequence's tiles, subtract the exp(0)=1
    contribution of the zero-padded slots from l, divide, transpose.

Layouts are pre-transposed on the host so every device DMA is one fully
contiguous block per tile and the PE never needs an on-chip transpose.
Per-tile input row layout (128 partitions x 2080 bf16):
  cols [0,1024):     K^T   (partition=d, col=kh*128+s)
  cols [1024,1056):  q^T   (partition=d, col=kh*4+j), pre-scaled by 1/sqrt(D)
  cols [1056,2080):  V     (partition=s, col=kh*128+d)
Per-tile output row layout (128 partitions x 64 f32):
  cols [0,32):  o_tile (partition=d, col=kh*4+j), unnormalized
  row 0, cols [32,64):  l_tile (sum of exp weights per (kh,j))
"""

import math
import os

import numpy as np

B, H, KVH, D = 32, 32, 8, 128
G = H // KVH
BLOCK_SIZE = 16
MAX_BLOCKS = 256
NUM_BLOCKS = B * MAX_BLOCKS
MAX_KV = MAX_BLOCKS * BLOCK_SIZE
SCALE = 0.08838834764831845
NCORES = 8
TILE = 128

KV_DTYPE = os.environ.get("BASS_KV_DTYPE", "bfloat16")

X_COLS = KVH * TILE + KVH * D + H  # 2080
O_OFF = 0
L_OFF = H  # in the [128, 64] output tile, l lives at row 0, cols [32,64)

LAST_RESULT = None  # BassKernelResults of the most recent run (for test.py)

_NC_CACHE = {}


def _install_trace_shim():
    """Register the axon NTFF profile hook (missing from the stub antenv) and
    stub the S3 artifact upload, so trace=True yields exec_time_ns."""
    import sys
    import types

    if "antenv.axon_hooks" not in sys.modules:
        mod = types.ModuleType("antenv.axon_hooks")
        _hook = [None]
        mod.set_axon_ntff_profile_hook = lambda h: _hook.__setitem__(0, h)
        mod.get_axon_ntff_profile_hook = lambda: _hook[0]
        sys.modules["antenv.axon_hooks"] = mod
        import antenv

        antenv.axon_hooks = mod
    from antenv.axon_hooks import (
        get_axon_ntff_profile_hook,
        set_axon_ntff_profile_hook,
    )

    if get_axon_ntff_profile_hook() is None:
        try:
            from trn_agent_boot.trn_boot import _ntff_profile_via_ctypes

            set_axon_ntff_profile_hook(
                _ntff_profile_via_ctypes("/opt/axon/libaxon_pjrt.so")
            )
        except Exception:
            pass
    import concourse.bass_utils as bu

    bu.upload_artifacts = lambda tmpdir: f"file://{tmpdir}"


def _install_ldw_opt_patch():
    """Experiment: flip walrus --enable-ldw-opt to true (default pipeline passes
    false). Gated by BASS_LDW_OPT=1."""
    import concourse.bass_utils as bu

    orig = bu.run_command

    def patched(argv, **kwargs):
        argv = [
            a.replace("--enable-ldw-opt=false", "--enable-ldw-opt=true")
            if isinstance(a, str)
            else a
            for a in argv
        ]
        return orig(argv, **kwargs)

    if getattr(bu.run_command, "__name__", "") != "patched":
        bu.run_command = patched


def _build_nc(n_t: int, dt_name: str):
    import concourse.mybir as mybir
    import concourse.tile as tile
    from concourse import bacc

    key = (n_t, dt_name)
    if key in _NC_CACHE:
        return _NC_CACHE[key]

    DT = getattr(mybir.dt, dt_name)
    F32 = mybir.dt.float32

    assert n_t % 4 == 0
    nc = bacc.Bacc("TRN2", target_bir_lowering=False, num_devices=NCORES)
    x = nc.dram_tensor("x", [n_t, TILE, X_COLS], DT, kind="ExternalInput")
    # y batches 4 tiles per row-block so output DMA rows are 1 KiB, not 256 B
    y = nc.dram_tensor("y", [n_t // 4, TILE, 8 * H], F32, kind="ExternalOutput")

    with tile.TileContext(nc) as tc:
        with (
            tc.tile_pool(name="consts", bufs=1) as consts,
            tc.tile_pool(name="kq", bufs=8) as kq_pool,
            tc.tile_pool(name="vp", bufs=8) as v_pool,
            tc.tile_pool(name="wt", bufs=4) as wt_pool,
            tc.tile_pool(name="outs", bufs=4) as out_pool,
            tc.tile_pool(name="ps_sc", bufs=3, space="PSUM") as ps_sc,
            tc.tile_pool(name="ps_o", bufs=3, space="PSUM") as ps_o,
            tc.tile_pool(name="ps_l", bufs=2, space="PSUM") as ps_l,
        ):
            ones = consts.tile([TILE, 1], DT)
            nc.vector.memset(ones, 1.0)

            NKQ = KVH * TILE + H  # 1056
            y_sb = None
            for t in range(n_t):
                # split each tile across the two HWDGE rings: K+q feeds QK as
                # soon as it lands, V only gates the PV half
                kq_t = kq_pool.tile([TILE, NKQ], DT)
                nc.sync.dma_start(out=kq_t, in_=x[t][:, :NKQ])
                v_t = v_pool.tile([TILE, KVH * D], DT)
                nc.scalar.dma_start(out=v_t, in_=x[t][:, NKQ:])

                sc = ps_sc.tile([TILE, H], F32)
                for kh in range(KVH):
                    nc.tensor.matmul(
                        sc[:, kh * G:(kh + 1) * G],
                        lhsT=kq_t[:, kh * TILE:(kh + 1) * TILE],
                        rhs=kq_t[:, KVH * TILE + kh * G:KVH * TILE + (kh + 1) * G],
                        start=(kh == 0),
                        stop=(kh == KVH - 1),
                    )
                w_t = wt_pool.tile([TILE, H], DT)
                nc.scalar.activation(w_t, sc, mybir.ActivationFunctionType.Exp)

                o_ps = ps_o.tile([D, H], F32)
                for kh in range(KVH):
                    nc.tensor.matmul(
                        o_ps[:, kh * G:(kh + 1) * G],
                        lhsT=v_t[:, kh * D:(kh + 1) * D],
                        rhs=w_t[:, kh * G:(kh + 1) * G],
                        start=(kh == 0),
                        stop=(kh == KVH - 1),
                    )
                l_ps = ps_l.tile([1, H], F32)
                nc.tensor.matmul(l_ps, lhsT=ones, rhs=w_t, start=True, stop=True)

                if t % 4 == 0:
                    y_sb = out_pool.tile([TILE, 8 * H], F32)
                off = (t % 4) * 2 * H
                nc.vector.tensor_copy(y_sb[:, off:off + H], o_ps)
                nc.vector.tensor_copy(y_sb[0:1, off + H:off + 2 * H], l_ps)
                if t % 4 == 3:
                    nc.gpsimd.dma_start(out=y[t // 4], in_=y_sb)
    nc.finalize()
    _NC_CACHE[key] = nc
    return nc


def kernel(q, k, v, k_cache, v_cache, block_tables, context_lens, slot_mapping):
    global LAST_RESULT
    from concourse.bass_utils import run_bass_kernel_spmd

    trace = bool(os.environ.get("BASS_TRACE"))
    if trace:
        _install_trace_shim()
    if os.environ.get("BASS_LDW_OPT"):
        _install_ldw_opt_patch()

    q = np.asarray(q, dtype=np.float32)
    k = np.asarray(k, dtype=np.float32)
    v = np.asarray(v, dtype=np.float32)
    k_cache = np.asarray(k_cache)
    v_cache = np.asarray(v_cache)
    block_tables = np.asarray(block_tables)
    context_lens = np.asarray(context_lens).astype(np.int64)
    slot_mapping = np.asarray(slot_mapping).astype(np.int64)

    # --- resolve paged layout -------------------------------------------------
    if np.array_equal(block_tables.ravel(), np.arange(NUM_BLOCKS, dtype=np.int64)):
        k_seq = k_cache.reshape(B, MAX_KV, KVH, D)  # zero-copy view
        v_seq = v_cache.reshape(B, MAX_KV, KVH, D)
        flat_pos = slot_mapping  # slot index == b*MAX_KV + pos under arange tables
    else:  # general fallback: true gather (slow, but correct for any table)
        k_seq = k_cache[block_tables].reshape(B, MAX_KV, KVH, D)
        v_seq = v_cache[block_tables].reshape(B, MAX_KV, KVH, D)
        blk = slot_mapping // BLOCK_SIZE
        off = slot_mapping % BLOCK_SIZE
        flat_pos = np.empty(B, np.int64)
        for b in range(B):
            tb = np.where(block_tables[b] == blk[b])[0][0]
            flat_pos[b] = b * MAX_KV + tb * BLOCK_SIZE + off[b]

    # --- tile map -------------------------------------------------------------
    ctx = context_lens.astype(np.int64)
    n_t_seq = [int(math.ceil(int(c) / TILE)) for c in ctx]
    seq_tile_start = np.concatenate([[0], np.cumsum(n_t_seq)]).astype(np.int64)
    g_tiles = int(seq_tile_start[-1])
    n_t = (g_tiles + NCORES - 1) // NCORES
    n_t = (n_t + 3) // 4 * 4  # y-batching works in groups of 4 tiles
    g_pad = n_t * NCORES

    if KV_DTYPE == "bfloat16":
        import ml_dtypes

        dt_np = ml_dtypes.bfloat16
    else:
        dt_np = np.float32

    x_g = np.zeros((g_pad, TILE, X_COLS), dt_np)
    KOFF, QOFF, VOFF = 0, KVH * TILE, KVH * TILE + H

    for b in range(B):
        c = int(ctx[b])
        t0 = int(seq_tile_start[b])
        nt = n_t_seq[b]
        kb = np.zeros((nt * TILE, KVH, D), np.float32)
        vb = np.zeros((nt * TILE, KVH, D), np.float32)
        kb[:c] = k_seq[b, :c]
        vb[:c] = v_seq[b, :c]
        # store_kvcache: new token for seq b lands at flat_pos[b] % MAX_KV
        p = int(flat_pos[b] - b * MAX_KV)
        if 0 <= p < c:
            kb[p] = k[b]
            vb[p] = v[b]
        # K^T tiles: [s, kh, d] -> [t, d, kh, s]
        kt = kb.reshape(nt, TILE, KVH, D).transpose(0, 3, 2, 1)
        x_g[t0:t0 + nt, :, KOFF:QOFF] = kt.reshape(nt, D, KVH * TILE).astype(dt_np)
        x_g[t0:t0 + nt, :, QOFF:VOFF] = (q[b].T * SCALE).astype(dt_np)[None]
        # V tiles: [t, s, kh*d]
        x_g[t0:t0 + nt, :, VOFF:] = vb.reshape(nt, TILE, KVH * D).astype(dt_np)

    in_maps = [{"x": x_g[c0 * n_t:(c0 + 1) * n_t]} for c0 in range(NCORES)]

    nc = _build_nc(n_t, KV_DTYPE)
    res = run_bass_kernel_spmd(
        nc, in_maps, core_ids=list(range(NCORES)), trace=trace
    )
    LAST_RESULT = res

    y_b = np.concatenate([res.results[c]["y"] for c in range(NCORES)], axis=0)
    # [G4, 128, 4*64] -> per-tile [G, 128, 64]
    y_all = (
        y_b.reshape(-1, TILE, 4, 2 * H).transpose(0, 2, 1, 3).reshape(-1, TILE, 2 * H)
    )

    out = np.empty((B, H, D), np.float32)
    for b in range(B):
        t0 = int(seq_tile_start[b])
        nt = n_t_seq[b]
        o_b = y_all[t0:t0 + nt, :, :H].sum(axis=0)       # [D, H]
        l_b = y_all[t0:t0 + nt, 0, H:].sum(axis=0)       # [H]
        l_b = l_b - (nt * TILE - int(ctx[b]))            # remove exp(0) pad terms
        out[b] = (o_b / l_b).T
    return out


# revision 17
# speedup vs baseline: 1.4766x; 1.1021x over previous
"""Paged-KV GQA decode attention on 8 TRN2 NeuronCores.

Strategy (data-parallel over flattened token tiles):
  * Host: resolve the paged cache (block_tables is a disjoint contiguous
    arange layout -> zero-copy reshape; general gather fallback otherwise),
    apply the store_kvcache update, slice each sequence's valid prefix
    [0, ctx_len), pad to 128-token tiles, and pack the global tile list.
  * The global tile stream is split contiguously across the 8 cores
    (perfect +-1 tile balance). Per tile the device computes, for each of
    the 8 KV heads, scoresT = K_tile^T @ qT (PE, stationary = K^T so scores
    land transposed [s, q]), w = exp(scoresT) (ACT, no max subtraction
    needed: |scores| <= ~6), o_tile = V_tile^T @ w (PE), and
    l_tile = ones^T @ w (PE). Per-tile unnormalized (o, l) go back to HBM.
  * Host: sum (o, l) over each sequence's tiles, subtract the exp(0)=1
    contribution of the zero-padded slots from l, divide, transpose.

Layouts are pre-transposed on the host so every device DMA is one fully
contiguous block per tile and the PE never needs an on-chip transpose.
Per-tile input row layout (128 partitions x 2080 bf16):
  cols [0,1024):     K^T   (partition=d, col=kh*128+s)
  cols [1024,1056):  q^T   (partition=d, col=kh*4+j), pre-scaled by 1/sqrt(D)
  cols [1056,2080):  V     (partition=s, col=kh*128+d)
Per-tile output row layout (128 partitions x 64 f32):
  cols [0,32):  o_tile (partition=d, col=kh*4+j), unnormalized
  row 0, cols [32,64):  l_tile (sum of exp weights per (kh,j))
"""

import math
import os

import numpy as np

B, H, KVH, D = 32, 32, 8, 128
G = H // KVH
BLOCK_SIZE = 16
MAX_BLOCKS = 256
NUM_BLOCKS = B * MAX_BLOCKS
MAX_KV = MAX_BLOCKS * BLOCK_SIZE
SCALE = 0.08838834764831845
NCORES = 8
TILE = 128

KV_DTYPE = os.environ.get("BASS_KV_DTYPE", "bfloat16")

X_COLS = KVH * TILE + KVH * D + H  # 2080
O_OFF = 0
L_OFF = H  # in the [128, 64] output tile, l lives at row 0, cols [32,64)

LAST_RESULT = None  # BassKernelResults of the most recent run (for test.py)

_NC_CACHE = {}


def _install_trace_shim():
    """Register the axon NTFF profile hook (missing from the stub antenv) and
    stub the S3 artifact upload, so trace=True yields exec_time_ns."""
    import sys
    import types

    if "antenv.axon_hooks" not in sys.modules:
        mod = types.ModuleType("antenv.axon_hooks")
        _hook = [None]
        mod.set_axon_ntff_profile_hook = lambda h: _hook.__setitem__(0, h)
        mod.get_axon_ntff_profile_hook = lambda: _hook[0]
        sys.modules["antenv.axon_hooks"] = mod
        import antenv

        antenv.axon_hooks = mod
    from antenv.axon_hooks import (
        get_axon_ntff_profile_hook,
        set_axon_ntff_profile_hook,
    )

    if get_axon_ntff_profile_hook() is None:
        try:
            from trn_agent_boot.trn_boot import _ntff_profile_via_ctypes

            set_axon_ntff_profile_hook(
                _ntff_profile_via_ctypes("/opt/axon/libaxon_pjrt.so")
            )
        except Exception:
            pass
    import concourse.bass_utils as bu

    bu.upload_artifacts = lambda tmpdir: f"file://{tmpdir}"


def _install_ldw_opt_patch():
    """Experiment: flip walrus --enable-ldw-opt to true (default pipeline passes
    false). Gated by BASS_LDW_OPT=1."""
    import concourse.bass_utils as bu

    orig = bu.run_command

    def patched(argv, **kwargs):
        argv = [
            a.replace("--enable-ldw-opt=false", "--enable-ldw-opt=true")
            if isinstance(a, str)
            else a
            for a in argv
        ]
        return orig(argv, **kwargs)

    if getattr(bu.run_command, "__name__", "") != "patched":
        bu.run_command = patched


def _build_nc(n_t: int, dt_name: str):
    import concourse.mybir as mybir
    import concourse.tile as tile
    from concourse import bacc

    key = (n_t, dt_name)
    if key in _NC_CACHE:
        return _NC_CACHE[key]

    DT = getattr(mybir.dt, dt_name)
    F32 = mybir.dt.float32

    assert n_t % 4 == 0
    nc = bacc.Bacc("TRN2", target_bir_lowering=False, num_devices=NCORES)
    x = nc.dram_tensor("x", [n_t, TILE, X_COLS], DT, kind="ExternalInput")
    # y batches 4 tiles per row-block so output DMA rows are 1 KiB, not 256 B
    y = nc.dram_tensor("y", [n_t // 4, TILE, 8 * H], F32, kind="ExternalOutput")

    with tile.TileContext(nc) as tc:
        with (
            tc.tile_pool(name="consts", bufs=1) as consts,
            tc.tile_pool(name="kq", bufs=12) as kq_pool,
            tc.tile_pool(name="vp", bufs=12) as v_pool,
            tc.tile_pool(name="wt", bufs=6) as wt_pool,
            tc.tile_pool(name="outs", bufs=6) as out_pool,
            tc.tile_pool(name="ps_sc", bufs=3, space="PSUM") as ps_sc,
            tc.tile_pool(name="ps_o", bufs=3, space="PSUM") as ps_o,
            tc.tile_pool(name="ps_l", bufs=2, space="PSUM") as ps_l,
        ):
            ones = consts.tile([TILE, 1], DT)
            nc.vector.memset(ones, 1.0)

            NKQ = KVH * TILE + H  # 1056
            y_sb = None
            for t in range(n_t):
                # split each tile across the two HWDGE rings: K+q feeds QK as
                # soon as it lands, V only gates the PV half
                kq_t = kq_pool.tile([TILE, NKQ], DT)
                nc.sync.dma_start(out=kq_t, in_=x[t][:, :NKQ])
                v_t = v_pool.tile([TILE, KVH * D], DT)
                nc.scalar.dma_start(out=v_t, in_=x[t][:, NKQ:])

                sc = ps_sc.tile([TILE, H], F32)
                for kh in range(KVH):
                    nc.tensor.matmul(
                        sc[:, kh * G:(kh + 1) * G],
                        lhsT=kq_t[:, kh * TILE:(kh + 1) * TILE],
                        rhs=kq_t[:, KVH * TILE + kh * G:KVH * TILE + (kh + 1) * G],
                        start=(kh == 0),
                        stop=(kh == KVH - 1),
                    )
                w_t = wt_pool.tile([TILE, H], DT)
                nc.scalar.activation(w_t, sc, mybir.ActivationFunctionType.Exp)

                o_ps = ps_o.tile([D, H], F32)
                for kh in range(KVH):
                    nc.tensor.matmul(
                        o_ps[:, kh * G:(kh + 1) * G],
                        lhsT=v_t[:, kh * D:(kh + 1) * D],
                        rhs=w_t[:, kh * G:(kh + 1) * G],
                        start=(kh == 0),
                        stop=(kh == KVH - 1),
                    )
                l_ps = ps_l.tile([1, H], F32)
                nc.tensor.matmul(l_ps, lhsT=ones, rhs=w_t, start=True, stop=True)

                if t % 4 == 0:
                    y_sb = out_pool.tile([TILE, 8 * H], F32)
                off = (t % 4) * 2 * H
                nc.vector.tensor_copy(y_sb[:, off:off + H], o_ps)
                nc.vector.tensor_copy(y_sb[0:1, off + H:off + 2 * H], l_ps)
                if t % 4 == 3:
                    nc.gpsimd.dma_start(out=y[t // 4], in_=y_sb)
    nc.finalize()
    _NC_CACHE[key] = nc
    return nc


def kernel(q, k, v, k_cache, v_cache, block_tables, context_lens, slot_mapping):
    global LAST_RESULT
    from concourse.bass_utils import run_bass_kernel_spmd

    trace = bool(os.environ.get("BASS_TRACE"))
    if trace:
        _install_trace_shim()
    if os.environ.get("BASS_LDW_OPT"):
        _install_ldw_opt_patch()

    q = np.asarray(q, dtype=np.float32)
    k = np.asarray(k, dtype=np.float32)
    v = np.asarray(v, dtype=np.float32)
    k_cache = np.asarray(k_cache)
    v_cache = np.asarray(v_cache)
    block_tables = np.asarray(block_tables)
    context_lens = np.asarray(context_lens).astype(np.int64)
    slot_mapping = np.asarray(slot_mapping).astype(np.int64)

    # --- resolve paged layout -------------------------------------------------
    if np.array_equal(block_tables.ravel(), np.arange(NUM_BLOCKS, dtype=np.int64)):
        k_seq = k_cache.reshape(B, MAX_KV, KVH, D)  # zero-copy view
        v_seq = v_cache.reshape(B, MAX_KV, KVH, D)
        flat_pos = slot_mapping  # slot index == b*MAX_KV + pos under arange tables
    else:  # general fallback: true gather (slow, but correct for any table)
        k_seq = k_cache[block_tables].reshape(B, MAX_KV, KVH, D)
        v_seq = v_cache[block_tables].reshape(B, MAX_KV, KVH, D)
        blk = slot_mapping // BLOCK_SIZE
        off = slot_mapping % BLOCK_SIZE
        flat_pos = np.empty(B, np.int64)
        for b in range(B):
            tb = np.where(block_tables[b] == blk[b])[0][0]
            flat_pos[b] = b * MAX_KV + tb * BLOCK_SIZE + off[b]

    # --- tile map -------------------------------------------------------------
    ctx = context_lens.astype(np.int64)
    n_t_seq = [int(math.ceil(int(c) / TILE)) for c in ctx]
    seq_tile_start = np.concatenate([[0], np.cumsum(n_t_seq)]).astype(np.int64)
    g_tiles = int(seq_tile_start[-1])
    n_t = (g_tiles + NCORES - 1) // NCORES
    n_t = (n_t + 3) // 4 * 4  # y-batching works in groups of 4 tiles
    g_pad = n_t * NCORES

    if KV_DTYPE == "bfloat16":
        import ml_dtypes

        dt_np = ml_dtypes.bfloat16
    else:
        dt_np = np.float32

    x_g = np.zeros((g_pad, TILE, X_COLS), dt_np)
    KOFF, QOFF, VOFF = 0, KVH * TILE, KVH * TILE + H

    for b in range(B):
        c = int(ctx[b])
        t0 = int(seq_tile_start[b])
        nt = n_t_seq[b]
        kb = np.zeros((nt * TILE, KVH, D), np.float32)
        vb = np.zeros((nt * TILE, KVH, D), np.float32)
        kb[:c] = k_seq[b, :c]
        vb[:c] = v_seq[b, :c]
        # store_kvcache: new token for seq b lands at flat_pos[b] % MAX_KV
        p = int(flat_pos[b] - b * MAX_KV)
        if 0 <= p < c:
            kb[p] = k[b]
            vb[p] = v[b]
        # K^T tiles: [s, kh, d] -> [t, d, kh, s]
        kt = kb.reshape(nt, TILE, KVH, D).transpose(0, 3, 2, 1)
        x_g[t0:t0 + nt, :, KOFF:QOFF] = kt.reshape(nt, D, KVH * TILE).astype(dt_np)
        x_g[t0:t0 + nt, :, QOFF:VOFF] = (q[b].T * SCALE).astype(dt_np)[None]
        # V tiles: [t, s, kh*d]
        x_g[t0:t0 + nt, :, VOFF:] = vb.reshape(nt, TILE, KVH * D).astype(dt_np)

    in_maps = [{"x": x_g[c0 * n_t:(c0 + 1) * n_t]} for c0 in range(NCORES)]

    nc = _build_nc(n_t, KV_DTYPE)
    res = run_bass_kernel_spmd(
        nc, in_maps, core_ids=list(range(NCORES)), trace=trace
    )
    LAST_RESULT = res

    y_b = np.concatenate([res.results[c]["y"] for c in range(NCORES)], axis=0)
    # [G4, 128, 4*64] -> per-tile [G, 128, 64]
    y_all = (
        y_b.reshape(-1, TILE, 4, 2 * H).transpose(0, 2, 1, 3).reshape(-1, TILE, 2 * H)
    )

    out = np.empty((B, H, D), np.float32)
    for b in range(B):
        t0 = int(seq_tile_start[b])
        nt = n_t_seq[b]
        o_b = y_all[t0:t0 + nt, :, :H].sum(axis=0)       # [D, H]
        l_b = y_all[t0:t0 + nt, 0, H:].sum(axis=0)       # [H]
        l_b = l_b - (nt * TILE - int(ctx[b]))            # remove exp(0) pad terms
        out[b] = (o_b / l_b).T
    return out
